# revision 1
# baseline (speedup 1.0000x reference)
"""DetectPeaks (nms_detection) Trainium2 Bass kernel, v2.

Single-read design: x streams into SBUF once and stays resident; output
y is int8 (host widens to int32).  Per chunk of R=2 strip rows:

  vertical 3-max   Bv = max(r0, r0+1); Mv0 = max(above, Bv);
                   Mv1 = max(Bv, below)     [shared-pair: 1.5 ops/elem]
  horizontal 3-max interleaved even/odd via pair tree  [1.5 ops/elem]
  xE = (x >= M ? x : 0) in place            [tau-free!]
  OUT = (xE >= tau) -> int8                 [only post-collective pass]

Engine split: the Pool engine cannot run max/compare TensorTensor ops
(only add/sub/mult TTs + arbitrary TensorScalars), so Pool handles the
vertical maxes via max(a,b) = a + relu(b-a) (TT-sub, relu, TT-add) with
the relu on the otherwise-idle Activation engine.  DVE keeps the
strided horizontal tree, the xE select, the min/max reduction and OUT.
The global min/max reduction is two TSP-accum passes fully overlapped
with the load DMA; only OUT waits on the AllReduce.
"""

from contextlib import ExitStack

import numpy as np

import concourse.bacc as bacc
import concourse.bass as bass
import concourse.mybir as mybir
import concourse.tile as tile
from concourse import bass_isa
from concourse._compat import get_trn_type

F32 = mybir.dt.float32
I8 = mybir.dt.int8
Alu = mybir.AluOpType
AxX = mybir.AxisListType.X
THRESH = 0.1
FMAX = 3.4e38

P = 128


def _register_xsel_op():
    """out = in0 if in0 >= in1 else 0  (xE: x where x is the window max)."""
    import concourse.dve_ops as dve_ops
    from concourse.dve_spec import Spec, Src0, Src1, Zero, select, lower
    from concourse.dve_ops import has_src1, DveOpSpec

    name = "XSEL_ANT"
    for o in dve_ops.OPS:
        if o.name == name:
            return o

    def _ref(in0, in1, s0, s1, imm2):
        in0 = np.asarray(in0, np.float32)
        in1 = np.asarray(in1, np.float32).reshape(in0.shape)
        return np.where(in0 >= in1, in0, np.float32(0.0))

    spec = Spec(body=select(Src0 >= Src1, Src0, Zero), reference=_ref)
    row = dve_ops._CUSTOM_DVE_ROW_BASE + len(dve_ops.OPS)
    shas = {}
    for ver in ("v3", "v4"):
        tmp = DveOpSpec(name=name, opcode=row, uops=lower(spec, ver=ver),
                        rd1_en=has_src1(spec))
        shas[ver] = tmp.sha(ver)
    op = dve_ops.DveOp(name, spec, subdim=False, uops_sha=shas)
    dve_ops.OPS.append(op)
    dve_ops._SUB_OPCODE_FOR_NAME[name] = row
    dve_ops.CUSTOM_DVE_SPECS[name] = spec
    return op


def flat(ap):
    return ap.rearrange("p a w -> p (a w)")


def m3(ap):
    return ap.rearrange("p r c one -> p r (c one)")


def build_nc(rows, W, ncores, R=2, lc_rows=2, cv=800, act_relu=True,
             sc_at=8, debug=False):
    """Per-core SPMD program.

    rows:  rows of the per-core shard (bpc square images stacked)
    W:     image width (even)
    cv:    vertical-pass columns [0, cv) computed directly on DVE;
           [cv, W) via sub/relu/add on Pool(+Act)
    act_relu: relu of the decomposed max on Activation (else Pool TSP)
    sc_at: pool-chunk index at which the collective is emitted
    """
    rp = rows // P
    assert R == 2 and rp % R == 0 and rp % lc_rows == 0 and W % 2 == 0
    npc = rp // R
    nlc = rp // lc_rows
    FA = lc_rows * W
    Wh = W // 2
    H = W
    ppi = H // rp
    nimg = P // ppi

    global _XSEL_OP
    _XSEL_OP = _register_xsel_op()

    nc = bacc.Bacc(
        get_trn_type() or "TRN2",
        target_bir_lowering=False,
        debug=debug,
        num_devices=ncores,
    )

    x = nc.dram_tensor("x", [rows, W], F32, kind="ExternalInput")
    y = nc.dram_tensor("y", [rows, W], I8, kind="ExternalOutput")
    cc_in = nc.dram_tensor("cc_in", [1, 2], F32)
    cc_out = nc.dram_tensor("cc_out", [1, 2], F32, addr_space="Shared")

    xf = x[:].rearrange("(p a) w -> p (a w)", p=P)   # [128, rp*W]

    with tile.TileContext(nc) as tc:
        with ExitStack() as ctx:
            st = ctx.enter_context(tc.tile_pool(name="st", bufs=1))

            X = st.tile([P, rp * W], F32)
            HT = st.tile([P, W], F32)     # x strip-row -1 (prev partition)
            HB = st.tile([P, W], F32)     # x strip-row rp (next partition)
            Xv = X[:].rearrange("p (r w) -> p r w", w=W)
            Xq = X[:].rearrange("p (r c t) -> p r c t", t=2, c=Wh)

            # halo DMAs (independent of the chunk loads, issued first)
            nc.sync.dma_start(
                HT[1:P, :],
                bass.AP(x, (rp - 1) * W, [[rp * W, P - 1], [1, W]]))
            nc.sync.dma_start(HT[0:1, :], bass.AP(x, 0, [[W, 1], [1, W]]))

            # loads + min/max TSP-accum per chunk (out = in, unchanged)
            maxs = st.tile([P, nlc], F32)
            mins = st.tile([P, nlc], F32)
            bhmax = st.tile([P, max(1, npc - 3)], F32)
            RBt = st.tile([nimg, W], F32)
            RBb = st.tile([nimg, W], F32)

            def emit_load(lc):
                if lc == 1:
                    # HB is only needed by the last pool chunk; deferring
                    # it lets load chunk 0 land sooner
                    nc.sync.dma_start(
                        HB[0:P - 1, :],
                        bass.AP(x, rp * W, [[rp * W, P - 1], [1, W]]))
                    nc.sync.dma_start(
                        HB[P - 1:P, :],
                        bass.AP(x, (rows - 1) * W, [[W, 1], [1, W]]))
                sl = X[:, lc * FA:(lc + 1) * FA]
                nc.sync.dma_start(sl, xf[:, lc * FA:(lc + 1) * FA])
                nc.vector.tensor_scalar(
                    sl, sl, FMAX, None, op0=Alu.min, op1=Alu.min,
                    accum_out=mins[:, lc:lc + 1])
                if lc >= nlc - 3:
                    # rows of the last pool chunks (whose Bh accumulators
                    # would be emitted after the collective) get a direct
                    # full-width max reduction instead
                    nc.vector.tensor_scalar(
                        sl, sl, -FMAX, None, op0=Alu.max, op1=Alu.max,
                        accum_out=maxs[:, lc:lc + 1])

            def emit_rb_saves(top):
                # save original image top/bottom rows (pre-xE)
                for k in range(nimg):
                    if top:
                        nc.sync.dma_start(RBt[k:k + 1, :],
                                          X[k * ppi:k * ppi + 1, 0:W])
                    else:
                        pb_ = (k + 1) * ppi - 1
                        nc.sync.dma_start(
                            RBb[k:k + 1, :],
                            X[pb_:pb_ + 1, (rp - 1) * W:rp * W])

            # persistent small tiles
            stat = st.tile([P, 8], F32)
            red = st.tile([P, 2], F32)
            gt = st.tile([1, 2], F32)
            g = st.tile([P, 2], F32)
            sc = st.tile([P, 8], F32)    # [gmin, d, r, tau, 0.1d]
            gmin_ap = sc[:, 0:1]
            r_ap = sc[:, 2:3]
            tau_ap = sc[:, 3:4]
            MB = st.tile([P, 6 * rp], F32)  # Mv cols 1,2,3,W-4,W-3,W-2
            MBs = MB[:].rearrange("p (s r) -> p s r", s=6)
            BL = st.tile([P, rp], F32)
            BR = st.tile([P, rp], F32)
            ORBt = st.tile([nimg, W], I8)
            ORBb = st.tile([nimg, W], I8)

            def emit_collective():
                nc.vector.tensor_reduce(
                    stat[:, 4:5], bhmax[:], axis=AxX, op=Alu.max)
                nc.vector.tensor_reduce(
                    stat[:, 5:6], maxs[:, nlc - 3:nlc], axis=AxX,
                    op=Alu.max)
                nc.vector.tensor_tensor(
                    stat[:, 0:1], stat[:, 4:5], stat[:, 5:6], op=Alu.max)
                nc.vector.tensor_reduce(
                    stat[:, 1:2], mins[:], axis=AxX, op=Alu.min)
                nc.gpsimd.tensor_copy(stat[:, 2:3], stat[:, 0:1])
                nc.gpsimd.tensor_scalar_mul(stat[:, 3:4], stat[:, 1:2], -1.0)
                nc.gpsimd.partition_all_reduce(
                    red[:], stat[:, 2:4], channels=P,
                    reduce_op=bass_isa.ReduceOp.max)
                nc.sync.dma_start(cc_in[:], red[0:1, :])
                if ncores > 1:
                    nc.gpsimd.collective_compute(
                        "AllReduce", Alu.max,
                        replica_groups=[list(range(ncores))],
                        ins=[cc_in[:]], outs=[cc_out[:]])
                    gsrc = cc_out
                else:
                    gsrc = cc_in
                nc.sync.dma_start(gt[:], gsrc[:])
                nc.gpsimd.partition_broadcast(g[:], gt[:], channels=P)
                nc.gpsimd.tensor_scalar_mul(sc[:, 0:1], g[:, 1:2], -1.0)
                nc.gpsimd.tensor_tensor(sc[:, 1:2], g[:, 0:1], g[:, 1:2],
                                        op=Alu.add)           # d = gmax-gmin
                nc.gpsimd.tensor_scalar(sc[:, 4:5], sc[:, 1:2], THRESH, None,
                                        op0=Alu.mult)         # 0.1*d
                nc.gpsimd.tensor_tensor(sc[:, 3:4], sc[:, 4:5], sc[:, 0:1],
                                        op=Alu.add)           # tau
                nc.gpsimd.tensor_scalar_mul(sc[:, 5:6], sc[:, 3:4], -1.0)

            def emit_borders(lo, hi):
                nr = hi - lo
                if lo == 0:
                    nc.vector.reciprocal(sc[:, 2:3], sc[:, 1:2])  # r = 1/d
                # border cols w=0 / w=W-1 for strip rows [lo, hi):
                #   OUT[h,0]   = hm(M(h,2))   != hm(x[h,0])
                #   OUT[h,W-1] = hm(M(h,W-3)) != hm(x[h,W-1])
                t = st.tile([P, 4 * nr], F32, tag=f"bt{lo}")
                mb = lambda j: MB[:, j * rp + lo:j * rp + hi]
                nc.vector.tensor_tensor(t[:, 0:nr], mb(0), mb(1), op=Alu.max)
                nc.vector.tensor_tensor(
                    t[:, nr:2 * nr], t[:, 0:nr], mb(2), op=Alu.max)  # M2L
                nc.vector.tensor_tensor(
                    t[:, 2 * nr:3 * nr], mb(3), mb(4), op=Alu.max)
                nc.vector.tensor_tensor(
                    t[:, 3 * nr:4 * nr], t[:, 2 * nr:3 * nr], mb(5),
                    op=Alu.max)                                      # M2R
                # hm(v) = (q >= c)*q with q = (v-gmin)*r (ref rounding)
                q = st.tile([P, 4 * nr], F32, tag=f"bq{lo}")
                m = st.tile([P, 4 * nr], F32, tag=f"bm{lo}")
                hm = st.tile([P, 4 * nr], F32, tag=f"bh{lo}")
                srcs = [flat(Xv[:, lo:hi, 0:1]), flat(Xv[:, lo:hi, W - 1:W]),
                        t[:, nr:2 * nr], t[:, 3 * nr:4 * nr]]
                for i, s in enumerate(srcs):
                    nc.gpsimd.tensor_scalar(
                        q[:, i * nr:(i + 1) * nr], s, gmin_ap, r_ap,
                        op0=Alu.subtract, op1=Alu.mult)
                nc.gpsimd.tensor_scalar(m[:], q[:], THRESH, None,
                                        op0=Alu.is_ge)
                nc.gpsimd.tensor_tensor(hm[:], m[:], q[:], op=Alu.mult)
                nc.vector.tensor_tensor(
                    BL[:, lo:hi], hm[:, 2 * nr:3 * nr], hm[:, 0:nr],
                    op=Alu.not_equal)
                nc.vector.tensor_tensor(
                    BR[:, lo:hi], hm[:, 3 * nr:4 * nr], hm[:, nr:2 * nr],
                    op=Alu.not_equal)
                # border rows: OUT = (q(x) >= c) on the saved rows
                # (q computed in place over RB)
                RBx = RBt if lo == 0 else RBb
                ORBx = ORBt if lo == 0 else ORBb
                nc.gpsimd.tensor_scalar(
                    RBx[:], RBx[:], sc[0:nimg, 0:1], sc[0:nimg, 2:3],
                    op0=Alu.subtract, op1=Alu.mult)
                nc.gpsimd.tensor_scalar(ORBx[:], RBx[:], THRESH, None,
                                        op0=Alu.is_ge)

            with tc.tile_pool(name="pv", bufs=2) as pv, \
                    tc.tile_pool(name="ph", bufs=2) as ph, \
                    tc.tile_pool(name="po", bufs=4) as po:

                def pmax(dst, a, b, tag):
                    """dst = max(a, b) on Pool(+Act): a + relu(b - a)."""
                    D = pv.tile([P, W - cv], F32, tag="D")
                    nc.gpsimd.tensor_tensor(D[:], b, a, op=Alu.subtract)
                    if act_relu:
                        nc.scalar.activation(
                            D[:], D[:], mybir.ActivationFunctionType.Relu)
                    else:
                        nc.gpsimd.tensor_scalar(D[:], D[:], 0.0, None,
                                                op0=Alu.max)
                    nc.gpsimd.tensor_tensor(dst, a, D[:], op=Alu.add)

                def emit_pool(pc):
                    r0 = R * pc
                    row = lambda r: X[:, r * W:(r + 1) * W]
                    above = HT[:, 0:W] if pc == 0 else row(r0 - 1)
                    below = HB[:, 0:W] if pc == npc - 1 else row(r0 + R)
                    # vertical shared-pair 3-max
                    Bv = pv.tile([P, W], F32, tag="Bv")
                    Mv = pv.tile([P, R * W], F32, tag="Mv")
                    Mv3 = Mv[:].rearrange("p (a w) -> p a w", w=W)
                    Mvq = Mv[:].rearrange("p (r c t) -> p r c t", t=2, c=Wh)
                    if cv > 0:
                        nc.vector.tensor_tensor(
                            Bv[:, 0:cv], row(r0)[:, 0:cv],
                            row(r0 + 1)[:, 0:cv], op=Alu.max)
                        nc.vector.tensor_tensor(
                            Mv[:, 0:cv], above[:, 0:cv], Bv[:, 0:cv],
                            op=Alu.max)
                        nc.vector.tensor_tensor(
                            Mv[:, W:W + cv], Bv[:, 0:cv], below[:, 0:cv],
                            op=Alu.max)
                    if cv < W:
                        pmax(Bv[:, cv:W], row(r0)[:, cv:W],
                             row(r0 + 1)[:, cv:W], "dBv")
                        pmax(Mv[:, cv:W], above[:, cv:W], Bv[:, cv:W],
                             "dMv0")
                        pmax(Mv[:, W + cv:2 * W], Bv[:, cv:W],
                             below[:, cv:W], "dMv1")
                    # horizontal interleaved pair tree (DVE, strided)
                    #   Bh[c] = max(Mv[2c], Mv[2c+1])            c=0..Wh-1
                    #   M(k,0) = even col 2k+2 = max(Mv[2k+1], Bh[k+1])
                    #   M(k,1) = odd  col 2k+1 = max(Bh[k], Mv[2k+2])
                    Bh = ph.tile([P, R * Wh], F32, tag="Bh")
                    Bh4 = Bh[:].rearrange("p (r c one) -> p r c one",
                                          one=1, c=Wh)
                    nc.vector.tensor_tensor(
                        Bh4[:, :, :, :], Mvq[:, :, :, 0:1],
                        Mvq[:, :, :, 1:2], op=Alu.max)
                    if pc < npc - 3:
                        # Bh covers every Mv element and Mv covers every x
                        # element, so max over Bh = max over x (half the
                        # elements of a direct reduction)
                        nc.vector.tensor_scalar(
                            Bh[:], Bh[:], -FMAX, None, op0=Alu.max,
                            op1=Alu.max, accum_out=bhmax[:, pc:pc + 1])
                    M = ph.tile([P, R * (Wh - 1) * 2], F32, tag="M")
                    M5 = M[:].rearrange("p (r c t) -> p r c t",
                                        t=2, c=Wh - 1)
                    nc.vector.tensor_tensor(
                        M5[:, :, :, 0:1], Mvq[:, :, 0:Wh - 1, 1:2],
                        Bh4[:, :, 1:Wh, :], op=Alu.max)
                    nc.vector.tensor_tensor(
                        M5[:, :, :, 1:2], Bh4[:, :, 0:Wh - 1, :],
                        Mvq[:, :, 1:Wh, 0:1], op=Alu.max)
                    # border-col extracts (transposed into col-major MB)
                    nc.vector.tensor_copy(
                        MBs[:, 0:3, r0:r0 + R],
                        Mv3[:, :, 1:4].rearrange("p r c -> p c r"))
                    nc.vector.tensor_copy(
                        MBs[:, 3:6, r0:r0 + R],
                        Mv3[:, :, W - 4:W - 1].rearrange("p r c -> p c r"))
                    return M5

                def emit_xe(pc, M5):
                    r0 = R * pc
                    # even cols 2..W-2: Xq[.., k+1, 0] vs M5[.., k, 0]
                    xe = m3(Xq[:, r0:r0 + R, 1:Wh, 0:1])
                    nc.vector._custom_dve(
                        _XSEL_OP, out=xe, in0=xe,
                        in1=m3(M5[:, :, :, 0:1]), s0=0.0, s1=0.0)
                    # odd cols 1..W-3: Xq[.., k, 1] vs M5[.., k, 1]
                    xo = m3(Xq[:, r0:r0 + R, 0:Wh - 1, 1:2])
                    nc.vector._custom_dve(
                        _XSEL_OP, out=xo, in0=xo,
                        in1=m3(M5[:, :, :, 1:2]), s0=0.0, s1=0.0)

                def emit_out(pc, on_act=True):
                    r0 = R * pc
                    OUT = po.tile([P, R * W], I8, tag="OUT")
                    OUT3 = OUT[:].rearrange("p (r w) -> p r w", w=W)
                    if on_act:
                        # sign(x - tau) in {-1,0,1}; host takes == 1
                        nc.scalar.activation(
                            OUT[:], X[:, r0 * W:(r0 + R) * W],
                            mybir.ActivationFunctionType.Sign,
                            bias=sc[:, 5:6])
                    else:
                        nc.vector.tensor_scalar(
                            OUT[:], X[:, r0 * W:(r0 + R) * W], tau_ap, None,
                            op0=Alu.is_gt)
                    nc.vector.tensor_copy(
                        flat(OUT3[:, :, 0:1]), BL[:, r0:r0 + R])
                    nc.vector.tensor_copy(
                        flat(OUT3[:, :, W - 1:W]), BR[:, r0:r0 + R])
                    if pc == 0:
                        for k in range(nimg):
                            nc.sync.dma_start(
                                OUT[k * ppi:k * ppi + 1, 0:W],
                                ORBt[k:k + 1, :])
                    if pc == npc - 1:
                        for k in range(nimg):
                            pb_ = (k + 1) * ppi - 1
                            nc.sync.dma_start(
                                OUT[pb_:pb_ + 1, (R - 1) * W:R * W],
                                ORBb[k:k + 1, :])
                    nc.sync.dma_start(
                        bass.AP(y, r0 * W, [[rp * W, P], [1, R * W]]),
                        OUT[:])

                pend_xe = []
                pend_out = []
                half = max(npc // 2, min(npc - 4, (3 * npc) // 4))
                state = {"emitted": 0, "bA": False, "sc": False,
                         "n_out": 0}

                def drain_out(lim):
                    while pend_out and pend_out[0] < lim:
                        i = state["n_out"]
                        emit_out(pend_out.pop(0), on_act=(i % 2 == 0))
                        state["n_out"] = i + 1

                def pump(upto):
                    while state["emitted"] < min(upto, npc):
                        pc = state["emitted"]
                        M5 = emit_pool(pc)
                        pend_xe.append((pc, M5))
                        if len(pend_xe) > 1:
                            pcx, M5x = pend_xe.pop(0)
                            emit_xe(pcx, M5x)
                            pend_out.append(pcx)
                        state["emitted"] += 1
                        if (pc >= half + 2 and state["sc"]
                                and not state["bA"]):
                            # rows [0, half*R) of Mv/xE are complete and
                            # the collective is already emitted, so the
                            # border ops are data-ready when reached
                            emit_borders(0, half * R)
                            state["bA"] = True
                        if state["bA"]:
                            drain_out(half)

                for lc in range(nlc):
                    emit_load(lc)
                    if lc == 0:
                        emit_rb_saves(top=True)
                    if lc == nlc - 1:
                        emit_rb_saves(top=False)
                        # all min/max accum slots are now emitted; the
                        # stat reduces and the AllReduce go here so they
                        # sit deep enough in each queue to be data-ready
                        emit_collective()
                        state["sc"] = True
                    # pool chunk pc needs x rows up to 2*pc+2; load lc
                    # covers rows up to lc_rows*lc + lc_rows - 1
                    pump((lc_rows * lc + lc_rows - 3) // 2 + 1)
                pump(npc)
                if not state["bA"]:
                    emit_borders(0, half * R)
                    state["bA"] = True
                emit_borders(half * R, rp)
                for pcx, M5x in pend_xe:
                    emit_xe(pcx, M5x)
                    pend_out.append(pcx)
                for i, pcx in enumerate(pend_out):
                    emit_out(pcx, on_act=((state["n_out"] + i) % 2 == 0))

    nc.compile()
    return nc


_NC_CACHE = {}


def _get_nc(rows, W, ncores):
    key = (rows, W, ncores)
    if key not in _NC_CACHE:
        _NC_CACHE[key] = build_nc(rows, W, ncores)
    return _NC_CACHE[key]


def kernel(heatmap: np.ndarray) -> np.ndarray:
    from concourse.bass_utils import run_bass_kernel_spmd

    heatmap = np.asarray(heatmap)
    B, Cc, H, W = heatmap.shape
    ncores = 8
    bpc = B // ncores
    rows = bpc * H
    nc = _get_nc(rows, W, ncores)
    shards = heatmap.reshape(ncores, rows, W)
    in_maps = [{"x": np.ascontiguousarray(shards[c])} for c in range(ncores)]
    res = run_bass_kernel_spmd(nc, in_maps, list(range(ncores)))
    out = np.stack([res.results[c]["y"] for c in range(ncores)])
    return (out.reshape(B, Cc, H, W) == 1).astype(np.int32)

